# revision 51
# baseline (speedup 1.0000x reference)
"""TRN2 Bass kernel for nn_AttentionStoreProcessor (dense transformer attention).

Full (unsharded) inputs in, full output out. Internally:
  - CAPE rotation + softmax scale folded into Wq/Wk on host; heads padded
    20 -> 24, tensor-parallel 3 heads/core across 8 cores, partial outputs
    summed on host (pad heads contribute exactly zero).
  - All device operands fp16 (PSUM accumulation f32); rel-err ~1e-3 vs the
    2e-2 gate.
  - hs transposed DRAM->SBUF by the DMA XBAR engine (no PE transposes).
  - Scores [ktok, q] on PE; exp on ACT (the critical engine, kept exp-only);
    PV computed transposed ([q, hd] out) with u as the stationary operand so
    the moving dim is hd+1 (65, fused ones-column = softmax denominator)
    instead of q - roughly half the PE cost of attention.
  - Per-query softmax normalization is a per-partition scalar multiply.
  - Attention output re-transposed via XBAR DMA; output projection from the
    transposed layout; residual/bias/core-sum on host.
  - q processed in chunks (896, 768, 384), each as two passes (heads 0+1,
    then head 2); PV consumption lags scores by 12 kt steps (u tiles are
    cheap in SBUF) so PE never blocks on a fresh exp. Projection chunks 1-3
    and v-projection tiles are woven into chunk-0 attention as PE filler
    granules ordered by data deadline; outproj of chunk i fills chunk i+1;
    each chunk's head-2 finish work is deferred into the next chunk's first
    step so chunk boundaries overlap, and the last chunk drains via PE
    transposes to skip the XBAR round-trip latency.
"""
import numpy as np
from contextlib import ExitStack

import concourse.bacc as bacc
import concourse.mybir as mybir
import concourse.tile as tile
from concourse.bass_utils import run_bass_kernel_spmd

F32 = mybir.dt.float32
F16 = mybir.dt.float16
AF = mybir.ActivationFunctionType

HEADS = 20
PAD_HEADS = 24
HPC = 3  # heads per core
N_CORES = 8
S = 2048
D = 1280
HD = 64
L = 1024
KT = D // 128  # 10 contraction tiles
TOKT = S // 128  # 16 token tiles

QCHUNKS = [(0, 7), (896, 6), (1664, 3)]  # (qoff, n_qtiles)

_CACHED_NC = None


def _build_nc():
    nc = bacc.Bacc("TRN2", debug=False, num_devices=N_CORES)

    hs = nc.dram_tensor("hs", [S, D], F16, kind="ExternalInput").ap()
    wg = nc.dram_tensor("wg", [128, 6 * KT * 128], F16, kind="ExternalInput").ap()
    wv = nc.dram_tensor("wv", [128, KT * 192], F16, kind="ExternalInput").ap()
    wo = nc.dram_tensor("wo", [128, 2560], F16, kind="ExternalInput").ap()
    io = nc.dram_tensor("io", [128, 136], F16, kind="ExternalInput").ap()
    out = nc.dram_tensor("out", [S, D], F16, kind="ExternalOutput").ap()

    out_r = out.rearrange("(n p) d -> n p d", p=128)

    with (
        tile.TileContext(nc) as tc,
        ExitStack() as ctx,
        nc.allow_low_precision(reason="fp16 kernel; tolerance 2e-2"),
    ):
        persist = ctx.enter_context(tc.tile_pool(name="persist", bufs=1))

        # ---- small const + exp table warmup ----
        io_sb = persist.tile([128, 136], F16, tag="io")
        nc.sync.dma_start(io_sb[:], io[:])
        ones_sb = io_sb[:, 0:8]
        ident_sb = io_sb[:, 8:136]
        warm = persist.tile([128, 8], F16, tag="warm")
        nc.scalar.activation(warm[:], io_sb[:, 0:8], AF.Exp)

        # ---- loads (SP/HWDGE), ordered by first use ----
        wg_sb = persist.tile([128, 6 * KT * 128], F16, tag="wg")
        hsT3 = persist.tile([128, KT, S], F16, tag="hsT")
        wv_sb = persist.tile([128, KT * 192], F16, tag="wv")
        wo_sb = persist.tile([128, 2560], F16, tag="wo")
        for half in range(2):
            hh = slice(half * 1024, (half + 1) * 1024)
            if half == 0:
                nc.sync.dma_start(wg_sb[:, 0:2560], wg[:, 0:2560])
            for k in range(KT):
                nc.sync.dma_start_transpose(
                    hsT3[:, k, hh], hs[hh, k * 128 : (k + 1) * 128]
                )
            if half == 0:
                nc.sync.dma_start(wv_sb[:], wv[:])
        nc.sync.dma_start(wg_sb[:, 2560:7680], wg[:, 2560:7680])
        nc.sync.dma_start(wo_sb[:], wo[:])

        # ---- persistent SBUF state ----
        QA = persist.tile([128, S], F16, tag="QA")  # rows 0:64 q_h0, 64:128 q_h1
        KA = persist.tile([128, S], F16, tag="KA")  # rows 0:64 k_h0, 64:128 k_h1
        QK2 = persist.tile([128, S], F16, tag="QK2")  # rows 0:64 q2, 64:128 k2
        QB2 = persist.tile([128, S], F16, tag="QB2")  # rows 64:128 <- q2 shifted
        v_sb = persist.tile([128, TOKT, 195], F16, tag="v_sb")
        oT = persist.tile([128, 2 * TOKT, 128], F16, tag="oT")

        # ones columns of v (col 65h+64) give the softmax denominator in PV
        v4 = v_sb.rearrange("p t (h c) -> p t h c", h=3)
        nc.gpsimd.memset(v4[:, :, :, 64:65], 1.0)

        u_pool = tc.alloc_tile_pool(name="u", bufs=26)
        attn_pool = tc.alloc_tile_pool(name="attn", bufs=2)
        rc_pool = tc.alloc_tile_pool(name="rc", bufs=4)
        osb_pool = tc.alloc_tile_pool(name="osb", bufs=3)

        # psum: tag "sc" (3 x 2 banks) shared by scores/proj/vproj/outproj;
        # tags pvA/pvB (1 bank each) hold per-(pass,head) PV accumulators
        ps = tc.alloc_tile_pool(name="ps", bufs=1, space="PSUM")

        def sc_tile(name, w=1024, tag="sc"):
            return ps.tile([128, w], F32, tag="sc", bufs=3, name=name)

        def proj_mm(pp, ch, g, k):
            t = ch // 2
            base = (t * 3 + g) * 1280
            nc.tensor.matmul(
                pp[:, 0:512],
                wg_sb[:, base + k * 128 : base + (k + 1) * 128],
                hsT3[:, k, ch * 512 : (ch + 1) * 512],
                start=(k == 0),
                stop=(k == KT - 1),
            )

        def proj_group(ch, g, tag="sc"):
            dest = (QA, KA, QK2)[g]
            if tag == "pvB":
                pp = ps.tile([128, 512], F32, tag="pvB", name=f"pp{ch}_{g}")
            else:
                pp = sc_tile(f"pp{ch}_{g}", 512)
            for k in range(KT):
                proj_mm(pp, ch, g, k)
            nc.vector.tensor_copy(dest[:, ch * 512 : (ch + 1) * 512], pp[:, 0:512])

        def vproj(n):
            vp = sc_tile(f"vp{n}", 192)
            for k in range(KT):
                nc.tensor.matmul(
                    vp[:, 0:192],
                    hsT3[:, k, n * 128 : (n + 1) * 128],
                    wv_sb[:, k * 192 : (k + 1) * 192],
                    start=(k == 0),
                    stop=(k == KT - 1),
                )
            nc.vector.tensor_copy(
                v4[:, n, :, 0:64], vp[:, 0:192].rearrange("p (h c) -> p h c", h=3)
            )

        def head_ops(h):
            if h == 0:
                return KA, slice(0, 64), QA, slice(0, 64)
            if h == 1:
                return KA, slice(64, 128), QA, slice(64, 128)
            return QK2, slice(64, 128), QB2, slice(64, 128)

        def score_exp(qoff, cw, kt, h, name):
            ksrc, krows, qsrc, qrows = head_ops(h)
            sc = sc_tile(f"sc{name}")
            off = 0
            while off < cw:
                w = min(512, cw - off)
                nc.tensor.matmul(
                    sc[:, off : off + w],
                    ksrc[krows, kt * 128 : (kt + 1) * 128],
                    qsrc[qrows, qoff + off : qoff + off + w],
                    start=True,
                    stop=True,
                )
                off += w
            u = u_pool.tile([128, 1024], F16, tag="u", name=f"u{name}")
            nc.scalar.activation(u[:, 0:cw], sc[:, 0:cw], AF.Exp)
            return u

        def pvt(kt, h, qts, u, pv_h, first=True, last=True):
            # One accumulation group per (chunk, head) bank: start=True zeroes
            # the whole 2KB zero-region, so only the first matmul starts and
            # only the last one stops; intermediates accumulate in place.
            for qt in range(qts):
                nc.tensor.matmul(
                    pv_h[:, qt * 65 : (qt + 1) * 65],
                    u[:, qt * 128 : (qt + 1) * 128],
                    v_sb[:, kt, h * 65 : h * 65 + 65],
                    start=(kt == 0 and qt == 0 and first),
                    stop=(kt == TOKT - 1 and qt == qts - 1 and last),
                )

        def normalize(ci, h, qts, pv_h, attn, qt_range=None):
            if qt_range is None:
                rc = rc_pool.tile([128, 8], F32, tag="rc", name=f"rc{ci}_{h}")
                pvv = pv_h[:, 0 : qts * 65].rearrange("p (q c) -> p q c", q=qts)
                nc.vector.reciprocal(rc[:, 0:qts], pvv[:, :, 64:65])
                qt_range = range(qts)
            else:
                rc = qt_range[1]
                qt_range = qt_range[0]
            for qt in qt_range:
                nc.vector.tensor_scalar_mul(
                    attn[:, qt, h * 64 : (h + 1) * 64],
                    pv_h[:, qt * 65 : qt * 65 + 64],
                    rc[:, qt : qt + 1],
                )
            return rc

        _osb = {}

        def outproj_part(n, part, name, evac_eng="v"):
            if n not in _osb:
                _osb[n] = osb_pool.tile([128, D], F16, tag="osb", name=f"ob{n}")
            if part == 0:
                op = sc_tile(f"op{name}a", 1024)
                for half in range(2):
                    cs = slice(half * 512, (half + 1) * 512)
                    nc.tensor.matmul(
                        op[:, cs], oT[:, 2 * n, :], wo_sb[:, cs],
                        start=True, stop=False,
                    )
                    nc.tensor.matmul(
                        op[:, cs],
                        oT[0:64, 2 * n + 1, :],
                        wo_sb[0:64, 1280 + half * 512 : 1280 + (half + 1) * 512],
                        start=False,
                        stop=True,
                    )
                if evac_eng == "v":
                    nc.vector.tensor_copy(_osb[n][:, 0:1024], op[:, 0:1024])
                else:
                    nc.scalar.copy(_osb[n][:, 0:1024], op[:, 0:1024])
            else:
                op = sc_tile(f"op{name}b", 256)
                nc.tensor.matmul(
                    op[:, 0:256], oT[:, 2 * n, :], wo_sb[:, 1024:1280],
                    start=True, stop=False,
                )
                nc.tensor.matmul(
                    op[:, 0:256],
                    oT[0:64, 2 * n + 1, :],
                    wo_sb[0:64, 2304:2560],
                    start=False,
                    stop=True,
                )
                nc.vector.tensor_copy(_osb[n][:, 1024:1280], op[:, 0:256])
                nc.sync.dma_start(out_r[n], _osb[n][:])
                del _osb[n]

        def qb2_shift(ch):
            nc.sync.dma_start(
                QB2[64:128, ch * 512 : (ch + 1) * 512],
                QK2[0:64, ch * 512 : (ch + 1) * 512],
            )

        # ---- head: first three proj groups k-interleaved (paced by hsT DMA),
        # then the first v tiles; attention chunk 0 follows immediately
        pp_a = sc_tile("pp0_0", 512)
        pp_b = sc_tile("pp1_0", 512)
        pp_c = sc_tile("pp0_1", 512)
        for k in range(KT):
            proj_mm(pp_a, 0, 0, k)
            proj_mm(pp_b, 1, 0, k)
            proj_mm(pp_c, 0, 1, k)
        nc.vector.tensor_copy(QA[:, 0:512], pp_a[:, 0:512])
        nc.scalar.copy(QA[:, 512:1024], pp_b[:, 0:512])
        nc.scalar.copy(KA[:, 0:512], pp_c[:, 0:512])
        # granules for chunk 0, ordered by deadline:
        #   pass 1 (steps 0-15): vp_n before PVT(kt=n) at step n+1;
        #     KA ch1/2/3 before scores kt 4/8/12; QK2+QB2 ch0,ch1 by pass 2
        #   pass 2 (steps 16-31): QK2+QB2 ch2 before kt8, ch3 before kt12
        g1 = [
            [("v", 0)],
            [("v", 1)],
            [("v", 2), ("p", 1, 1)],
            [("v", 3)],
            [("v", 4)],
            [("v", 5)],
            [("v", 6), ("p", 2, 1)],
            [("v", 7)],
            [("v", 8)],
            [("v", 9)],
            [("v", 10), ("p", 3, 1)],
            [("v", 11), ("p", 0, 2)],
            [("v", 12), ("p", 1, 2)],
            [("v", 13), ("q2", 0)],
            [("v", 14), ("q2", 1)],
            [("v", 15)],
        ]
        g2 = [
            [("P", 2, 2)],
            [("q2", 2)],
            [("P", 2, 0)],
            [("P", 3, 2)],
            [("q2", 3)],
            [("P", 3, 0)],
        ]

        def run_granule(g):
            if g[0] == "v":
                vproj(g[1])
            elif g[0] == "p":
                proj_group(g[1], g[2])
            elif g[0] == "P":
                proj_group(g[1], g[2], tag="pvB")
            elif g[0] == "q2":
                qb2_shift(g[1])
            else:  # ("o", n, part)
                outproj_part(g[1], g[2], f"f{g[1]}")

        def attention(ci, fillers, finish_prev):
            """Emit chunk ci. fillers: per-step granule lists, consumed one
            per kt step across both passes (leftovers drain at the end).
            finish_prev: thunk emitted after this chunk's first kt step."""
            qoff, qts = QCHUNKS[ci]
            cw = qts * 128
            attn = attn_pool.tile([128, qts, 256], F16, tag="attn", name=f"at{ci}")
            nc.gpsimd.memset(attn[:, :, 192:256], 0)
            pvs = {}
            fi = 0
            for pi, heads in enumerate(((0, 1), (2,))):
                for j, h in enumerate(heads):
                    pvs[h] = ps.tile(
                        [128, 512], F32, tag="pvA" if j == 0 else "pvB",
                        name=f"pv{ci}_{h}",
                    )
                us = {}
                for kt in range(TOKT):
                    for h in heads:
                        us[(kt, h)] = score_exp(qoff, cw, kt, h, f"{ci}_{kt}_{h}")
                        if kt > 11:
                            pvt(kt - 12, h, qts, us.pop((kt - 12, h)), pvs[h])
                    if kt == 0 and pi == 0 and finish_prev is not None:
                        finish_prev()
                        finish_prev = None
                    if fi < len(fillers):
                        for g in fillers[fi]:
                            run_granule(g)
                        fi += 1
                for lag in (12, 11, 10, 9, 8, 7, 6, 5, 4, 3, 2):
                    for h in heads:
                        pvt(TOKT - lag, h, qts, us.pop((TOKT - lag, h)), pvs[h])
                for h in heads:
                    pvt(TOKT - 1, h, qts, us.pop((TOKT - 1, h)), pvs[h])
                    if pi == 0:
                        normalize(ci, h, qts, pvs[h], attn)
            while fi < len(fillers):
                for g in fillers[fi]:
                    run_granule(g)
                fi += 1

            def finish():
                # head-2 normalize + per-qtile attn transpose: each outproj
                # tile's oT dependency resolves as soon as its qtile drains.
                # The last chunk transposes on PE (short latency) instead of
                # the XBAR DMA (~3us issue+transfer+sem chain).
                rc = rc_pool.tile([128, 8], F32, tag="rc", name=f"rcf{ci}")
                pvv = pvs[2][:, 0 : qts * 65].rearrange("p (q c) -> p q c", q=qts)
                nc.vector.reciprocal(rc[:, 0:qts], pvv[:, :, 64:65])
                qtg = qoff // 128
                for qt in range(qts):
                    nc.vector.tensor_scalar_mul(
                        attn[:, qt, 128:192],
                        pvs[2][:, qt * 65 : qt * 65 + 64],
                        rc[:, qt : qt + 1],
                    )
                    if ci < 2:
                        nc.sync.dma_start_transpose(
                            oT[:, 2 * (qtg + qt) : 2 * (qtg + qt) + 2, :],
                            attn[:, qt, :],
                        )
                    else:
                        m = qtg + qt
                        t1 = ps.tile([128, 128], F16, tag="sc", bufs=3,
                                     name=f"tr1_{m}")
                        nc.tensor.matmul(
                            t1[:], attn[:, qt, 0:128], ident_sb,
                            is_transpose=True, start=True, stop=True,
                        )
                        t2 = ps.tile([64, 128], F16, tag="sc", bufs=3,
                                     name=f"tr2_{m}")
                        nc.tensor.matmul(
                            t2[:], attn[:, qt, 128:192], ident_sb,
                            is_transpose=True, start=True, stop=True,
                        )
                        nc.vector.tensor_copy(oT[:, 2 * m, :], t1[:])
                        nc.vector.tensor_copy(oT[0:64, 2 * m + 1, :], t2[:])

            return finish, [qoff // 128 + i for i in range(qts)]

        fin0, tiles0 = attention(0, g1 + g2, None)
        # ops wait the per-qtile attnT pipeline: 3 empty lead-in steps
        op0 = [[], [], []] + [[("o", n, p)] for n in tiles0 for p in (0, 1)]
        fin1, tiles1 = attention(1, op0, fin0)
        op1 = [[], [], []] + [[("o", n, p)] for n in tiles1 for p in (0, 1)]
        fin2, tiles2 = attention(2, op1, fin1)

        # tail: outproj evacs use the now-idle ACT
        fin2()
        for n in tiles2:
            outproj_part(n, 0, f"t{n}", evac_eng="s")
            outproj_part(n, 1, f"t{n}")

        osb_pool.release()
        rc_pool.release()
        attn_pool.release()
        u_pool.release()
        ps.release()

    nc.compile()
    return nc


def _get_nc():
    global _CACHED_NC
    if _CACHED_NC is None:
        _CACHED_NC = _build_nc()
    return _CACHED_NC


def _fold_cape(W, P):
    """W @ blockdiag(P) for 4x4 P repeated along channels: exact CAPE fold."""
    d = W.shape[1]
    W4 = W.reshape(W.shape[0], d // 4, 4)
    return np.einsum("cik,kj->cij", W4, P, optimize=True).reshape(W.shape[0], d)


def _klayout(W, cols):
    # [1280, cols] -> [128, KT*cols] with ktile-major free dim
    return np.ascontiguousarray(
        W.reshape(KT, 128, cols).transpose(1, 0, 2).reshape(128, KT * cols)
    )


def _f16(x):
    return np.ascontiguousarray(x.astype(np.float16))


def _prep_in_maps(hidden_states, p_out, p_out_inv, Wq, Wk, Wv, Wo):
    scale = HD ** -0.5
    hs2 = np.ascontiguousarray(hidden_states.reshape(S, D), dtype=np.float32)

    FEAT = PAD_HEADS * HD  # 1536
    Wq_eff = np.zeros((2, D, FEAT), np.float32)
    Wk_eff = np.zeros((2, D, FEAT), np.float32)
    for t in range(2):
        Wq_eff[t, :, :D] = _fold_cape(Wq, p_out_inv[0, t]) * scale
        Wk_eff[t, :, :D] = _fold_cape(Wk, p_out[0, t])
    Wv_pad = np.zeros((D, FEAT), np.float32)
    Wv_pad[:, :D] = Wv
    Wo_pad = np.zeros((FEAT, D), np.float32)
    Wo_pad[:D, :] = Wo

    hs_f16 = _f16(hs2)
    io = np.concatenate(
        [np.ones((128, 8), np.float32), np.eye(128, dtype=np.float32)], axis=1
    )
    in_maps = []
    for c in range(N_CORES):
        A = c * HPC * HD
        blocks = []
        for t in range(2):
            blocks.append(_klayout(Wq_eff[t][:, A : A + 128], 128))
            blocks.append(_klayout(Wk_eff[t][:, A : A + 128], 128))
            blocks.append(
                _klayout(
                    np.concatenate(
                        [
                            Wq_eff[t][:, A + 128 : A + 192],
                            Wk_eff[t][:, A + 128 : A + 192],
                        ],
                        axis=1,
                    ),
                    128,
                )
            )
        wgl = np.concatenate(blocks, axis=1)
        wvl = _klayout(Wv_pad[:, A : A + 192], 192)
        wol = np.concatenate(
            [
                Wo_pad[A : A + 128, :],
                np.concatenate(
                    [Wo_pad[A + 128 : A + 192, :], np.zeros((64, D), np.float32)],
                    axis=0,
                ),
            ],
            axis=1,
        )
        in_maps.append(
            {
                "hs": hs_f16,
                "wg": _f16(wgl),
                "wv": _f16(wvl),
                "wo": _f16(wol),
                "io": _f16(io),
            }
        )
    return in_maps


def kernel(hidden_states, p_out, p_out_inv, Wq, Wk, Wv, Wo, bo):
    hidden_states = np.asarray(hidden_states, dtype=np.float32)
    in_maps = _prep_in_maps(
        hidden_states,
        np.asarray(p_out, np.float32),
        np.asarray(p_out_inv, np.float32),
        np.asarray(Wq, np.float32),
        np.asarray(Wk, np.float32),
        np.asarray(Wv, np.float32),
        np.asarray(Wo, np.float32),
    )
    nc = _get_nc()
    res = run_bass_kernel_spmd(nc, in_maps, core_ids=list(range(N_CORES)))
    acc = np.zeros((S, D), np.float32)
    for c in range(N_CORES):
        acc += np.asarray(res.results[c]["out"], dtype=np.float32)
    acc += np.asarray(bo, np.float32)[None, :]
    out = acc.reshape(2, L, D) + hidden_states
    return out


# revision 61
# speedup vs baseline: 1.0224x; 1.0224x over previous
"""TRN2 Bass kernel for nn_AttentionStoreProcessor (dense transformer attention).

Full (unsharded) inputs in, full output out. Internally:
  - CAPE rotation + softmax scale folded into Wq/Wk on host; heads padded
    20 -> 24, tensor-parallel 3 heads/core across 8 cores, partial outputs
    summed on host (pad heads contribute exactly zero).
  - All device operands fp16 (PSUM accumulation f32); rel-err ~1e-3 vs the
    2e-2 gate.
  - hs transposed DRAM->SBUF by the DMA XBAR engine (no PE transposes).
  - Scores [ktok, q] on PE; exp on ACT (the critical engine, kept exp-only);
    PV computed transposed ([q, hd] out) with u as the stationary operand so
    the moving dim is hd+1 (65, fused ones-column = softmax denominator)
    instead of q - roughly half the PE cost of attention.
  - Per-query softmax normalization is a per-partition scalar multiply.
  - Attention output re-transposed via XBAR DMA; output projection from the
    transposed layout; residual/bias/core-sum on host.
  - q processed in chunks (896, 768, 384), each as two passes (heads 0+1,
    then head 2); PV consumption lags scores by 12 kt steps (u tiles are
    cheap in SBUF) so PE never blocks on a fresh exp. Projection chunks 1-3
    and v-projection tiles are woven into chunk-0 attention as PE filler
    granules ordered by data deadline; outproj of chunk i fills chunk i+1;
    each chunk's head-2 finish work is deferred into the next chunk's first
    step so chunk boundaries overlap, and the last chunk drains via PE
    transposes to skip the XBAR round-trip latency.
"""
import numpy as np
from contextlib import ExitStack

import concourse.bacc as bacc
import concourse.mybir as mybir
import concourse.tile as tile
from concourse.bass_utils import run_bass_kernel_spmd

F32 = mybir.dt.float32
F16 = mybir.dt.float16
AF = mybir.ActivationFunctionType

HEADS = 20
PAD_HEADS = 24
HPC = 3  # heads per core
N_CORES = 8
S = 2048
D = 1280
HD = 64
L = 1024
KT = D // 128  # 10 contraction tiles
TOKT = S // 128  # 16 token tiles

QCHUNKS = [(0, 7), (896, 6), (1664, 3)]  # (qoff, n_qtiles)

_CACHED_NC = None


def _build_nc():
    nc = bacc.Bacc("TRN2", debug=False, num_devices=N_CORES)

    hs = nc.dram_tensor("hs", [S, D], F16, kind="ExternalInput").ap()
    wg = nc.dram_tensor("wg", [128, 6 * KT * 128], F16, kind="ExternalInput").ap()
    wv = nc.dram_tensor("wv", [128, KT * 192], F16, kind="ExternalInput").ap()
    wo = nc.dram_tensor("wo", [128, 2560], F16, kind="ExternalInput").ap()
    io = nc.dram_tensor("io", [128, 136], F16, kind="ExternalInput").ap()
    out = nc.dram_tensor("out", [S, D], F16, kind="ExternalOutput").ap()

    out_r = out.rearrange("(n p) d -> n p d", p=128)

    with (
        tile.TileContext(nc) as tc,
        ExitStack() as ctx,
        nc.allow_low_precision(reason="fp16 kernel; tolerance 2e-2"),
    ):
        persist = ctx.enter_context(tc.tile_pool(name="persist", bufs=1))

        # ---- small const + exp table warmup ----
        io_sb = persist.tile([128, 136], F16, tag="io")
        nc.sync.dma_start(io_sb[:], io[:])
        ones_sb = io_sb[:, 0:8]
        ident_sb = io_sb[:, 8:136]
        warm = persist.tile([128, 8], F16, tag="warm")
        nc.scalar.activation(warm[:], io_sb[:, 0:8], AF.Exp)

        # ---- loads (SP/HWDGE), ordered by first use ----
        wg_sb = persist.tile([128, 6 * KT * 128], F16, tag="wg")
        hsT3 = persist.tile([128, KT, S], F16, tag="hsT")
        wv_sb = persist.tile([128, KT * 192], F16, tag="wv")
        wo_sb = persist.tile([128, 2560], F16, tag="wo")
        nc.sync.dma_start(wv_sb[:], wv[:])
        nc.sync.dma_start(wg_sb[:, 0:2560], wg[:, 0:2560])
        for k in range(KT):
            nc.sync.dma_start_transpose(
                hsT3[:, k, 0:1024], hs[0:1024, k * 128 : (k + 1) * 128]
            )
        for k in range(0, KT, 5):
            nc.sync.dma_start_transpose(
                hsT3[:, k : k + 5, 1024:2048],
                hs[1024:2048, k * 128 : (k + 5) * 128],
            )
        nc.sync.dma_start(wg_sb[:, 2560:7680], wg[:, 2560:7680])
        nc.sync.dma_start(wo_sb[:], wo[:])

        # ---- persistent SBUF state ----
        QA = persist.tile([128, S], F16, tag="QA")  # rows 0:64 q_h0, 64:128 q_h1
        KA = persist.tile([128, S], F16, tag="KA")  # rows 0:64 k_h0, 64:128 k_h1
        QK2 = persist.tile([128, S], F16, tag="QK2")  # rows 0:64 q2, 64:128 k2
        QB2 = persist.tile([128, S], F16, tag="QB2")  # rows 64:128 <- q2 shifted
        v_sb = persist.tile([128, TOKT, 195], F16, tag="v_sb")
        oT = persist.tile([128, 2 * TOKT, 128], F16, tag="oT")

        # ones columns of v (col 65h+64) give the softmax denominator in PV
        v4 = v_sb.rearrange("p t (h c) -> p t h c", h=3)
        nc.gpsimd.memset(v4[:, :, :, 64:65], 1.0)

        u_pool = tc.alloc_tile_pool(name="u", bufs=26)
        attn_pool = tc.alloc_tile_pool(name="attn", bufs=2)
        rc_pool = tc.alloc_tile_pool(name="rc", bufs=4)
        osb_pool = tc.alloc_tile_pool(name="osb", bufs=3)

        # psum: tag "sc" (3 x 2 banks) shared by scores/proj/vproj/outproj;
        # tags pvA/pvB (1 bank each) hold per-(pass,head) PV accumulators
        ps = tc.alloc_tile_pool(name="ps", bufs=1, space="PSUM")

        def sc_tile(name, w=1024, tag="sc"):
            return ps.tile([128, w], F32, tag="sc", bufs=3, name=name)

        def proj_mm(pp, ch, g, k):
            t = ch // 2
            base = (t * 3 + g) * 1280
            nc.tensor.matmul(
                pp[:, 0:512],
                wg_sb[:, base + k * 128 : base + (k + 1) * 128],
                hsT3[:, k, ch * 512 : (ch + 1) * 512],
                start=(k == 0),
                stop=(k == KT - 1),
            )

        def proj_group(ch, g, tag="sc"):
            dest = (QA, KA, QK2)[g]
            if tag == "pvB":
                pp = ps.tile([128, 512], F32, tag="pvB", name=f"pp{ch}_{g}")
            else:
                pp = sc_tile(f"pp{ch}_{g}", 512)
            for k in range(KT):
                proj_mm(pp, ch, g, k)
            nc.vector.tensor_copy(dest[:, ch * 512 : (ch + 1) * 512], pp[:, 0:512])

        def vproj(n):
            vp = sc_tile(f"vp{n}", 192)
            for k in range(KT):
                nc.tensor.matmul(
                    vp[:, 0:192],
                    hsT3[:, k, n * 128 : (n + 1) * 128],
                    wv_sb[:, k * 192 : (k + 1) * 192],
                    start=(k == 0),
                    stop=(k == KT - 1),
                )
            nc.vector.tensor_copy(
                v4[:, n, :, 0:64], vp[:, 0:192].rearrange("p (h c) -> p h c", h=3)
            )

        def head_ops(h):
            if h == 0:
                return KA, slice(0, 64), QA, slice(0, 64)
            if h == 1:
                return KA, slice(64, 128), QA, slice(64, 128)
            return QK2, slice(64, 128), QB2, slice(64, 128)

        def score_exp(qoff, cw, kt, h, name):
            ksrc, krows, qsrc, qrows = head_ops(h)
            sc = sc_tile(f"sc{name}")
            off = 0
            while off < cw:
                w = min(512, cw - off)
                nc.tensor.matmul(
                    sc[:, off : off + w],
                    ksrc[krows, kt * 128 : (kt + 1) * 128],
                    qsrc[qrows, qoff + off : qoff + off + w],
                    start=True,
                    stop=True,
                )
                off += w
            u = u_pool.tile([128, 1024], F16, tag="u", name=f"u{name}")
            nc.scalar.activation(u[:, 0:cw], sc[:, 0:cw], AF.Exp)
            return u

        def pvt(kt, h, qts, u, pv_h, first=True, last=True):
            # One accumulation group per (chunk, head) bank: start=True zeroes
            # the whole 2KB zero-region, so only the first matmul starts and
            # only the last one stops; intermediates accumulate in place.
            for qt in range(qts):
                nc.tensor.matmul(
                    pv_h[:, qt * 65 : (qt + 1) * 65],
                    u[:, qt * 128 : (qt + 1) * 128],
                    v_sb[:, kt, h * 65 : h * 65 + 65],
                    start=(kt == 0 and qt == 0 and first),
                    stop=(kt == TOKT - 1 and qt == qts - 1 and last),
                )

        def normalize(ci, h, qts, pv_h, attn, qt_range=None):
            if qt_range is None:
                rc = rc_pool.tile([128, 8], F32, tag="rc", name=f"rc{ci}_{h}")
                pvv = pv_h[:, 0 : qts * 65].rearrange("p (q c) -> p q c", q=qts)
                nc.vector.reciprocal(rc[:, 0:qts], pvv[:, :, 64:65])
                qt_range = range(qts)
            else:
                rc = qt_range[1]
                qt_range = qt_range[0]
            for qt in qt_range:
                nc.vector.tensor_scalar_mul(
                    attn[:, qt, h * 64 : (h + 1) * 64],
                    pv_h[:, qt * 65 : qt * 65 + 64],
                    rc[:, qt : qt + 1],
                )
            return rc

        _osb = {}

        def outproj_part(n, part, name, evac_eng="v"):
            if n not in _osb:
                _osb[n] = osb_pool.tile([128, D], F16, tag="osb", name=f"ob{n}")
            if part == 0:
                op = sc_tile(f"op{name}a", 1024)
                for half in range(2):
                    cs = slice(half * 512, (half + 1) * 512)
                    nc.tensor.matmul(
                        op[:, cs], oT[:, 2 * n, :], wo_sb[:, cs],
                        start=True, stop=False,
                    )
                    nc.tensor.matmul(
                        op[:, cs],
                        oT[0:64, 2 * n + 1, :],
                        wo_sb[0:64, 1280 + half * 512 : 1280 + (half + 1) * 512],
                        start=False,
                        stop=True,
                    )
                if evac_eng == "v":
                    nc.vector.tensor_copy(_osb[n][:, 0:1024], op[:, 0:1024])
                else:
                    nc.scalar.copy(_osb[n][:, 0:1024], op[:, 0:1024])
            else:
                op = sc_tile(f"op{name}b", 256)
                nc.tensor.matmul(
                    op[:, 0:256], oT[:, 2 * n, :], wo_sb[:, 1024:1280],
                    start=True, stop=False,
                )
                nc.tensor.matmul(
                    op[:, 0:256],
                    oT[0:64, 2 * n + 1, :],
                    wo_sb[0:64, 2304:2560],
                    start=False,
                    stop=True,
                )
                nc.vector.tensor_copy(_osb[n][:, 1024:1280], op[:, 0:256])
                nc.sync.dma_start(out_r[n], _osb[n][:])
                del _osb[n]

        def qb2_shift(ch):
            nc.sync.dma_start(
                QB2[64:128, ch * 512 : (ch + 1) * 512],
                QK2[0:64, ch * 512 : (ch + 1) * 512],
            )

        # ---- head: first three proj groups k-interleaved (paced by hsT DMA),
        # then the first v tiles; attention chunk 0 follows immediately
        pp_a = sc_tile("pp0_0", 512)
        pp_b = sc_tile("pp1_0", 512)
        pp_c = sc_tile("pp0_1", 512)
        pp_d = ps.tile([128, 512], F32, tag="pvB", name="pp1_1")
        for k in range(KT):
            proj_mm(pp_a, 0, 0, k)
            proj_mm(pp_b, 1, 0, k)
            proj_mm(pp_c, 0, 1, k)
            proj_mm(pp_d, 1, 1, k)
        nc.vector.tensor_copy(QA[:, 0:512], pp_a[:, 0:512])
        nc.scalar.copy(QA[:, 512:1024], pp_b[:, 0:512])
        nc.scalar.copy(KA[:, 0:512], pp_c[:, 0:512])
        nc.vector.tensor_copy(KA[:, 512:1024], pp_d[:, 0:512])
        # granules for chunk 0, ordered by deadline:
        #   pass 1 (steps 0-15): vp_n before PVT(kt=n) at step n+1;
        #     KA ch1/2/3 before scores kt 4/8/12; QK2+QB2 ch0,ch1 by pass 2
        #   pass 2 (steps 16-31): QK2+QB2 ch2 before kt8, ch3 before kt12
        g1 = [
            [("v", 0)],
            [("v", 1)],
            [("v", 2)],
            [("v", 3)],
            [("v", 4)],
            [("v", 5)],
            [("v", 6), ("p", 2, 1)],
            [("v", 7)],
            [("v", 8)],
            [("v", 9)],
            [("v", 10), ("p", 3, 1)],
            [("v", 11), ("p", 0, 2)],
            [("v", 12), ("p", 1, 2)],
            [("v", 13), ("q2", 0)],
            [("v", 14), ("q2", 1)],
            [("v", 15)],
        ]
        g2 = [
            [("P", 2, 2)],
            [("q2", 2)],
            [("P", 2, 0)],
            [("P", 3, 2)],
            [("q2", 3)],
            [("P", 3, 0)],
        ]

        def run_granule(g):
            if g[0] == "v":
                vproj(g[1])
            elif g[0] == "p":
                proj_group(g[1], g[2])
            elif g[0] == "P":
                proj_group(g[1], g[2], tag="pvB")
            elif g[0] == "q2":
                qb2_shift(g[1])
            else:  # ("o", n, part)
                outproj_part(g[1], g[2], f"f{g[1]}")

        def attention(ci, fillers, finish_prev):
            """Emit chunk ci. fillers: per-step granule lists, consumed one
            per kt step across both passes (leftovers drain at the end).
            finish_prev: thunk emitted after this chunk's first kt step."""
            qoff, qts = QCHUNKS[ci]
            cw = qts * 128
            attn = attn_pool.tile([128, qts, 256], F16, tag="attn", name=f"at{ci}")
            nc.gpsimd.memset(attn[:, :, 192:256], 0)
            pvs = {}
            fi = 0
            for pi, heads in enumerate(((0, 1), (2,))):
                for j, h in enumerate(heads):
                    pvs[h] = ps.tile(
                        [128, 512], F32, tag="pvA" if j == 0 else "pvB",
                        name=f"pv{ci}_{h}",
                    )
                us = {}
                for kt in range(TOKT):
                    for h in heads:
                        us[(kt, h)] = score_exp(qoff, cw, kt, h, f"{ci}_{kt}_{h}")
                        if kt > 11:
                            pvt(kt - 12, h, qts, us.pop((kt - 12, h)), pvs[h])
                    if kt == 0 and pi == 0 and finish_prev is not None:
                        finish_prev()
                        finish_prev = None
                    if fi < len(fillers):
                        for g in fillers[fi]:
                            run_granule(g)
                        fi += 1
                for lag in (12, 11, 10, 9, 8, 7, 6, 5, 4, 3, 2):
                    for h in heads:
                        pvt(TOKT - lag, h, qts, us.pop((TOKT - lag, h)), pvs[h])
                for h in heads:
                    pvt(TOKT - 1, h, qts, us.pop((TOKT - 1, h)), pvs[h])
                    if pi == 0:
                        normalize(ci, h, qts, pvs[h], attn)
            while fi < len(fillers):
                for g in fillers[fi]:
                    run_granule(g)
                fi += 1

            def finish():
                # head-2 normalize + per-qtile attn transpose: each outproj
                # tile's oT dependency resolves as soon as its qtile drains.
                # The last chunk transposes on PE (short latency) instead of
                # the XBAR DMA (~3us issue+transfer+sem chain).
                rc = rc_pool.tile([128, 8], F32, tag="rc", name=f"rcf{ci}")
                pvv = pvs[2][:, 0 : qts * 65].rearrange("p (q c) -> p q c", q=qts)
                nc.vector.reciprocal(rc[:, 0:qts], pvv[:, :, 64:65])
                qtg = qoff // 128
                for qt in range(qts):
                    nc.vector.tensor_scalar_mul(
                        attn[:, qt, 128:192],
                        pvs[2][:, qt * 65 : qt * 65 + 64],
                        rc[:, qt : qt + 1],
                    )
                    if ci < 2:
                        nc.sync.dma_start_transpose(
                            oT[:, 2 * (qtg + qt) : 2 * (qtg + qt) + 2, :],
                            attn[:, qt, :],
                        )
                    else:
                        m = qtg + qt
                        t1 = ps.tile([128, 128], F16, tag="sc", bufs=3,
                                     name=f"tr1_{m}")
                        nc.tensor.matmul(
                            t1[:], attn[:, qt, 0:128], ident_sb,
                            is_transpose=True, start=True, stop=True,
                        )
                        t2 = ps.tile([64, 128], F16, tag="sc", bufs=3,
                                     name=f"tr2_{m}")
                        nc.tensor.matmul(
                            t2[:], attn[:, qt, 128:192], ident_sb,
                            is_transpose=True, start=True, stop=True,
                        )
                        nc.vector.tensor_copy(oT[:, 2 * m, :], t1[:])
                        nc.vector.tensor_copy(oT[0:64, 2 * m + 1, :], t2[:])

            return finish, [qoff // 128 + i for i in range(qts)]

        fin0, tiles0 = attention(0, g1 + g2, None)
        # ops wait the per-qtile attnT pipeline: 3 empty lead-in steps
        op0 = [[], [], []] + [[("o", n, p)] for n in tiles0 for p in (0, 1)]
        fin1, tiles1 = attention(1, op0, fin0)
        op1 = [[], [], []] + [[("o", n, p)] for n in tiles1 for p in (0, 1)]
        fin2, tiles2 = attention(2, op1, fin1)

        # tail: outproj evacs use the now-idle ACT
        fin2()
        for n in tiles2:
            outproj_part(n, 0, f"t{n}", evac_eng="s")
            outproj_part(n, 1, f"t{n}")

        osb_pool.release()
        rc_pool.release()
        attn_pool.release()
        u_pool.release()
        ps.release()

    nc.compile()
    return nc


def _get_nc():
    global _CACHED_NC
    if _CACHED_NC is None:
        _CACHED_NC = _build_nc()
    return _CACHED_NC


def _fold_cape(W, P):
    """W @ blockdiag(P) for 4x4 P repeated along channels: exact CAPE fold."""
    d = W.shape[1]
    W4 = W.reshape(W.shape[0], d // 4, 4)
    return np.einsum("cik,kj->cij", W4, P, optimize=True).reshape(W.shape[0], d)


def _klayout(W, cols):
    # [1280, cols] -> [128, KT*cols] with ktile-major free dim
    return np.ascontiguousarray(
        W.reshape(KT, 128, cols).transpose(1, 0, 2).reshape(128, KT * cols)
    )


def _f16(x):
    return np.ascontiguousarray(x.astype(np.float16))


def _prep_in_maps(hidden_states, p_out, p_out_inv, Wq, Wk, Wv, Wo):
    scale = HD ** -0.5
    hs2 = np.ascontiguousarray(hidden_states.reshape(S, D), dtype=np.float32)

    FEAT = PAD_HEADS * HD  # 1536
    Wq_eff = np.zeros((2, D, FEAT), np.float32)
    Wk_eff = np.zeros((2, D, FEAT), np.float32)
    for t in range(2):
        Wq_eff[t, :, :D] = _fold_cape(Wq, p_out_inv[0, t]) * scale
        Wk_eff[t, :, :D] = _fold_cape(Wk, p_out[0, t])
    Wv_pad = np.zeros((D, FEAT), np.float32)
    Wv_pad[:, :D] = Wv
    Wo_pad = np.zeros((FEAT, D), np.float32)
    Wo_pad[:D, :] = Wo

    hs_f16 = _f16(hs2)
    io = np.concatenate(
        [np.ones((128, 8), np.float32), np.eye(128, dtype=np.float32)], axis=1
    )
    in_maps = []
    for c in range(N_CORES):
        A = c * HPC * HD
        blocks = []
        for t in range(2):
            blocks.append(_klayout(Wq_eff[t][:, A : A + 128], 128))
            blocks.append(_klayout(Wk_eff[t][:, A : A + 128], 128))
            blocks.append(
                _klayout(
                    np.concatenate(
                        [
                            Wq_eff[t][:, A + 128 : A + 192],
                            Wk_eff[t][:, A + 128 : A + 192],
                        ],
                        axis=1,
                    ),
                    128,
                )
            )
        wgl = np.concatenate(blocks, axis=1)
        wvl = _klayout(Wv_pad[:, A : A + 192], 192)
        wol = np.concatenate(
            [
                Wo_pad[A : A + 128, :],
                np.concatenate(
                    [Wo_pad[A + 128 : A + 192, :], np.zeros((64, D), np.float32)],
                    axis=0,
                ),
            ],
            axis=1,
        )
        in_maps.append(
            {
                "hs": hs_f16,
                "wg": _f16(wgl),
                "wv": _f16(wvl),
                "wo": _f16(wol),
                "io": _f16(io),
            }
        )
    return in_maps


def kernel(hidden_states, p_out, p_out_inv, Wq, Wk, Wv, Wo, bo):
    hidden_states = np.asarray(hidden_states, dtype=np.float32)
    in_maps = _prep_in_maps(
        hidden_states,
        np.asarray(p_out, np.float32),
        np.asarray(p_out_inv, np.float32),
        np.asarray(Wq, np.float32),
        np.asarray(Wk, np.float32),
        np.asarray(Wv, np.float32),
        np.asarray(Wo, np.float32),
    )
    nc = _get_nc()
    res = run_bass_kernel_spmd(nc, in_maps, core_ids=list(range(N_CORES)))
    acc = np.zeros((S, D), np.float32)
    for c in range(N_CORES):
        acc += np.asarray(res.results[c]["out"], dtype=np.float32)
    acc += np.asarray(bo, np.float32)[None, :]
    out = acc.reshape(2, L, D) + hidden_states
    return out
